# revision 32
# baseline (speedup 1.0000x reference)
"""GCN (2-layer GraphConv + ReLU + log_softmax) on 8 Trainium2 NeuronCores.

Strategy (graph/data parallel, per sharding hint):
  - Nodes are padded to 50176 = 8 * 49 * 128 and sharded contiguously:
    core c owns nodes [c*6272, (c+1)*6272).
  - Edges are routed to the core owning their *destination* node, sorted by
    destination tile (128 nodes), split into A/B groups by source index
    (src < 32768 vs >= 32768, so gather indices fit int16), and padded to
    128-edge chunks per (tile, group).
  - Gathered feature rows use a bf16 hi/lo split packed into one 256B row:
    row = [bf16(v) (64/40 cols) | bf16(v - hi) | pad]. The segment-sum
    matmul streams both halves through the PE in a single bf16 op
    (selector one-hot stationary, exact in bf16) and the two halves are
    re-added afterwards -- f32-quality precision (~2^-18 rel) at bf16
    matmul speed, with the same 256B/row gather wire cost as f32.
  - Gathers round-robin the 4 SWDGE queues (a single queue stalls the
    gather ucode on ring drain: 8.3us vs 2.9us per 1024-row call).
  - Layer 1 per core: dma_gather x[src] rows from the replicated packed x
    table in HBM -> per-chunk one-hot bf16 selector (DVE is_equal) ->
    one PE matmul per chunk accumulating aggT[128d, hi|lo] in PSUM ->
    hi+lo add -> PE transpose -> projections (f32) -> ReLU -> hT.
  - The layer-2 "rel" projection is applied *before* the second gather
    (segment_sum is linear): hp = h @ W2_rel.T per tile, split hi/lo bf16,
    packed to [NPC, 128] bf16, AllGathered (1.6MB/rank). Layer-2 root term
    hroot = h @ W2_root.T + b2 stays resident in SBUF.
  - Layer 2 per core: dma_gather hp rows, same selector matmul into
    acc2[128d, 40hi|40lo], hi+lo+hroot adds, log_softmax, DMA out.
"""

import numpy as np

# ---------------------------------------------------------------- problem cfg

P = 128


class Cfg:
    def __init__(self, n, e, in_ch, hid, out_ch, n_cores, tiles_per_core, split,
                 seg_tiles=7):
        self.N = n
        self.E = e
        self.IN = in_ch
        self.HID = hid
        self.OUT = out_ch
        self.NCORES = n_cores
        self.TPC = tiles_per_core
        self.NPC = tiles_per_core * P
        self.NPAD = self.NPC * n_cores
        self.SPLIT = split
        self.SEG = seg_tiles
        assert self.NPAD >= self.N
        assert self.SPLIT % P == 0


# SPLIT sits on a segment boundary (4 segs * 8 cores * 896 rows = 28672) so
# the A-table collective covers exactly segs 0-3 and fires before layer 1
# finishes; it must also stay <= 32768 so A-group gather indices fit int16.
REAL_CFG = Cfg(n=50000, e=800000, in_ch=64, hid=128, out_ch=40, n_cores=8,
               tiles_per_core=49, split=28672, seg_tiles=7)


def seg_perm(cfg):
    """Permutation: global node id -> segment-major gather-table row.

    Table rows are ordered [segment][core][tile-in-seg][node-in-tile] so a
    per-segment AllGather (each core contributing its 7-tile slice) writes a
    contiguous block, letting the collective overlap layer-1 compute.
    """
    n = np.arange(cfg.NPAD)
    c, rem = n // cfg.NPC, n % cfg.NPC
    t, r = rem // P, rem % P
    s, ts = t // cfg.SEG, t % cfg.SEG
    segrows = cfg.SEG * P
    return (s * (cfg.NCORES * segrows) + c * segrows + ts * P + r)


# ---------------------------------------------------------------- host preproc

def _bf16_round(x):
    """Round f32 -> bf16 (round-to-nearest-even), keep as float32 bits."""
    u = x.view(np.uint32)
    r = (u + 0x7FFF + ((u >> 16) & 1)) & 0xFFFF0000
    return r.view(np.float32)


def pack_hilo(v, width):
    """[R, C] f32 -> [R, width] bf16 rows [hi(C) | lo(C) | pad]."""
    import ml_dtypes
    hi = _bf16_round(np.ascontiguousarray(v))
    lo = _bf16_round(np.ascontiguousarray(v - hi))
    out = np.zeros((v.shape[0], width), dtype=ml_dtypes.bfloat16)
    out[:, :v.shape[1]] = hi.astype(ml_dtypes.bfloat16)
    out[:, v.shape[1]:2 * v.shape[1]] = lo.astype(ml_dtypes.bfloat16)
    return out


def preprocess(x, edge_index, cfg):
    """Build per-core gather-index / selector tensors and the chunk structure.

    Returns (kA, kB, per_core) where kA/kB are per-tile chunk counts (uniform
    across cores; padded to the max) and per_core is a list of dicts of numpy
    arrays for each core's in_map.
    """
    N, E = cfg.N, cfg.E
    perm = seg_perm(cfg)
    src = perm[np.asarray(edge_index[0], dtype=np.int64)]  # permuted table row
    dst = np.asarray(edge_index[1], dtype=np.int64)

    gtile = dst // P                      # global dst tile in [0, NCORES*TPC)
    grp = (src >= cfg.SPLIT).astype(np.int64)
    key = gtile * 2 + grp
    order = np.argsort(key, kind="stable")
    src_s = src[order]
    key_s = key[order]

    nkeys = cfg.NCORES * cfg.TPC * 2
    bounds = np.searchsorted(key_s, np.arange(nkeys + 1))
    counts = np.diff(bounds).reshape(cfg.NCORES, cfg.TPC, 2)

    kA = np.maximum(np.ceil(counts[:, :, 0] / P).max(axis=0), 0).astype(int)
    kB = np.maximum(np.ceil(counts[:, :, 1] / P).max(axis=0), 0).astype(int)
    # every tile gets at least one chunk so the PSUM accumulate chain exists
    kA = np.maximum(kA, 1)

    dst_rel_s = (dst[order] % P).astype(np.float32)

    def build_group(c, g, kX):
        """Concatenate this core's per-tile edge lists for group g, padding
        each tile to kX[t]*128 edges with (idx=0, dst_rel=-1)."""
        idx_parts = []
        rel_parts = []
        for t in range(cfg.TPC):
            key_base = (c * cfg.TPC + t) * 2 + g
            lo, hi = bounds[key_base], bounds[key_base + 1]
            n_real = hi - lo
            n_slots = kX[t] * P
            assert n_real <= n_slots
            idx = np.zeros(n_slots, dtype=np.int16)
            rel = np.full(n_slots, -1.0, dtype=np.float32)
            if n_real:
                s = src_s[lo:hi]
                idx[:n_real] = (s - (cfg.SPLIT if g else 0)).astype(np.int16)
                rel[:n_real] = dst_rel_s[lo:hi]
            idx_parts.append(idx)
            rel_parts.append(rel)
        return np.concatenate(idx_parts), np.concatenate(rel_parts)

    def idx_layout(flat16):
        # dma_gather index layout: [128, n/16]; idx i at (i%16, i//16),
        # replicated across the 8 Q7 cores (partitions 16k+r == r).
        cols = flat16.reshape(-1, 16).T          # [16, cols]
        return np.ascontiguousarray(np.tile(cols, (8, 1)))  # [128, cols]

    def rel_layout(flat):
        return np.ascontiguousarray(flat.reshape(-1, P).T)  # [128, n_chunks]

    x_pad = np.zeros((cfg.NPAD, cfg.IN), dtype=np.float32)
    x_pad[:N] = np.asarray(x, dtype=np.float32)
    # gather table in segment-major permuted row order: row perm[n] = x[n]
    x_tab = pack_hilo(x_pad[np.argsort(perm)], P)  # [NPAD, 128] bf16 hi|lo

    per_core = []
    for c in range(cfg.NCORES):
        idxA, relA = build_group(c, 0, kA)
        idxB, relB = build_group(c, 1, kB)
        xT_own = np.ascontiguousarray(
            x_pad[c * cfg.NPC:(c + 1) * cfg.NPC].T)   # [IN, NPC]
        per_core.append(dict(
            x_tab=x_tab,
            xT_own=xT_own,
            idxA=idx_layout(idxA),
            idxB=idx_layout(idxB),
            drA=rel_layout(relA),
            drB=rel_layout(relB),
        ))
    return list(kA), list(kB), per_core


def make_weight_inputs(W1_rel, b1, W1_root, W2_rel, b2, W2_root, cfg):
    f = np.float32
    w2relT = np.asarray(W2_rel, f).T              # [HID, OUT]
    w2rootT = np.asarray(W2_root, f).T            # [HID, OUT]
    ident = np.eye(P, dtype=f)
    return dict(
        w1relT=np.ascontiguousarray(np.asarray(W1_rel, f).T),    # [IN, HID]
        w1rootT=np.ascontiguousarray(np.asarray(W1_root, f).T),  # [IN, HID]
        b1=np.asarray(b1, f).reshape(cfg.HID, 1).copy(),
        w2bothT=np.ascontiguousarray(
            np.concatenate([w2relT, w2rootT], axis=1)),          # [HID, 2*OUT]
        b2rep=np.tile(np.asarray(b2, f).reshape(1, cfg.OUT), (P, 1)).copy(),
        iota=np.tile(np.arange(P, dtype=f).reshape(1, P), (P, 1)).copy(),
        ident=ident,
    )


# ---------------------------------------------------------------- bass build

def build(cfg, kA, kB):
    import concourse.bacc as bacc
    import concourse.tile as tile
    from concourse import mybir

    f32 = mybir.dt.float32
    bf16 = mybir.dt.bfloat16
    i16 = mybir.dt.int16
    Alu = mybir.AluOpType
    Act = mybir.ActivationFunctionType

    IN, HID, OUT, TPC = cfg.IN, cfg.HID, cfg.OUT, cfg.TPC
    NAc, NBc = sum(kA), sum(kB)
    offA = np.concatenate([[0], np.cumsum(kA)]).astype(int)
    offB = np.concatenate([[0], np.cumsum(kB)]).astype(int)

    nc = bacc.Bacc("TRN2", target_bir_lowering=False, debug=False,
                   num_devices=cfg.NCORES, num_swdge_queues=4)

    x_tab = nc.dram_tensor("x_tab", [cfg.NPAD, P], bf16, kind="ExternalInput")
    xT_own_d = nc.dram_tensor("xT_own", [IN, cfg.NPC], f32, kind="ExternalInput")
    idxA_d = nc.dram_tensor("idxA", [P, NAc * 8], i16, kind="ExternalInput")
    idxB_d = nc.dram_tensor("idxB", [P, NBc * 8], i16, kind="ExternalInput")
    drA_d = nc.dram_tensor("drA", [P, NAc], f32, kind="ExternalInput")
    drB_d = nc.dram_tensor("drB", [P, NBc], f32, kind="ExternalInput")
    w1relT_d = nc.dram_tensor("w1relT", [IN, HID], f32, kind="ExternalInput")
    w1rootT_d = nc.dram_tensor("w1rootT", [IN, HID], f32, kind="ExternalInput")
    b1_d = nc.dram_tensor("b1", [HID, 1], f32, kind="ExternalInput")
    w2bothT_d = nc.dram_tensor("w2bothT", [HID, 2 * OUT], f32,
                               kind="ExternalInput")
    b2rep_d = nc.dram_tensor("b2rep", [P, OUT], f32, kind="ExternalInput")
    iota_d = nc.dram_tensor("iota", [P, P], f32, kind="ExternalInput")
    ident_d = nc.dram_tensor("ident", [P, P], f32, kind="ExternalInput")
    out_d = nc.dram_tensor("out", [cfg.NPC, OUT], f32, kind="ExternalOutput")

    segs = [(s, min(s + cfg.SEG, TPC)) for s in range(0, TPC, cfg.SEG)]

    with tile.TileContext(nc) as tc:
        with (
            tc.tile_pool(name="const", bufs=1) as cp,
            tc.tile_pool(name="dram", bufs=1, space="DRAM") as dp,
        ):
            # ---- resident constants
            iota_s = cp.tile([P, P], f32)
            nc.sync.dma_start(iota_s[:], iota_d[:])
            ident_s = cp.tile([P, P], f32)
            nc.sync.dma_start(ident_s[:], ident_d[:])
            w1relT_s = cp.tile([IN, HID], f32)
            nc.sync.dma_start(w1relT_s[:], w1relT_d[:])
            w1rootT_s = cp.tile([IN, HID], f32)
            nc.sync.dma_start(w1rootT_s[:], w1rootT_d[:])
            b1_s = cp.tile([HID, 1], f32)
            nc.sync.dma_start(b1_s[:], b1_d[:])
            w2bothT_s = cp.tile([HID, 2 * OUT], f32)
            nc.sync.dma_start(w2bothT_s[:], w2bothT_d[:])
            b2_s = cp.tile([P, OUT], f32)
            nc.sync.dma_start(b2_s[:], b2rep_d[:])
            xT_own_s = cp.tile([IN, cfg.NPC], f32)
            nc.sync.dma_start(xT_own_s[:], xT_own_d[:])
            idxA_s = cp.tile([P, NAc * 8], i16)
            nc.sync.dma_start(idxA_s[:], idxA_d[:])
            idxB_s = cp.tile([P, NBc * 8], i16)
            nc.sync.dma_start(idxB_s[:], idxB_d[:])
            drA_s = cp.tile([P, NAc], f32)
            nc.sync.dma_start(drA_s[:], drA_d[:])
            drB_s = cp.tile([P, NBc], f32)
            nc.sync.dma_start(drB_s[:], drB_d[:])
            hroots = cp.tile([P, TPC * OUT], f32)

            hp_local = dp.tile([cfg.NPC, P], bf16)
            hp_full = dp.tile([cfg.NPAD, P], bf16)

            maxA = max(offA[t1] - offA[t0] for t0, t1 in segs)
            maxB = max(offB[t1] - offB[t0] for t0, t1 in segs)

            gq = [0]
            GMAX = 8  # dma_gather ring limit: 1024 idx (8 chunks) per call
            kmaxA = max(kA)
            kmaxB = max(kB) if max(kB) else 1

            def gather_piece(G, gtab, idx_s, base, c0, c1):
                # round-robin the 4 SWDGE queues; each ring caps at
                # 1024 descriptors and a lone queue stalls the ucode
                # on ring drain.
                nc.gpsimd.dma_gather(
                    G[:, c0:c1, :], gtab,
                    idx_s[:, (base + c0) * 8:(base + c1) * 8],
                    (c1 - c0) * P, (c1 - c0) * P, P,
                    queue_num=gq[0] % 4)
                gq[0] += 1

            def emit_tile(t, width, GA, offga, GB, offgb, sp, ap,
                          consume_tile):
                """Selector build + segment-sum matmul chain for one tile."""
                nch = kA[t] + kB[t]
                acc = ap.tile([P, width], f32, tag="acc")
                ci = 0
                for g, G, off, soff, dr, kmax in (
                    (0, GA, offA[t] - offga, offA[t], drA_s, kmaxA),
                    (1, GB, offB[t] - offgb, offB[t], drB_s, kmaxB),
                ):
                    kk = kA[t] if g == 0 else kB[t]
                    if kk == 0:
                        continue
                    S = sp.tile([P, kmax, P], bf16, tag=f"S{g}")
                    nc.vector.tensor_tensor(
                        out=S[:, :kk, :],
                        in0=iota_s[:].unsqueeze(1)
                            .to_broadcast([P, kk, P]),
                        in1=dr[:, soff:soff + kk].unsqueeze(2)
                            .to_broadcast([P, kk, P]),
                        op=Alu.is_equal)
                    for j in range(kk):
                        # acc[d, hi|lo] += sum_e S[e,d] G[e,:]
                        # selector one-hot is exact in bf16;
                        # hi+lo re-add recovers f32 precision.
                        nc.tensor.matmul(
                            acc[:], lhsT=S[:, j, :],
                            rhs=G[:, off + j, :width],
                            start=(ci == 0),
                            stop=(ci == nch - 1))
                        ci += 1
                consume_tile(t, acc)

            def layer1(gtabA, gtabB, consume_tile, seg_order):
                with (
                    tc.tile_pool(name="G1", bufs=2) as gp,
                    tc.tile_pool(name="S1", bufs=4) as sp,
                    tc.tile_pool(name="agg1", bufs=2, space="PSUM") as ap,
                ):
                    for si in seg_order:
                        t0, t1 = segs[si]
                        a0, a1 = offA[t0], offA[t1]
                        b0, b1_ = offB[t0], offB[t1]
                        GA = gp.tile([P, maxA, P], bf16, tag="GA")
                        for c0 in range(0, a1 - a0, GMAX):
                            gather_piece(GA, gtabA, idxA_s, a0,
                                         c0, min(c0 + GMAX, a1 - a0))
                        GB = gp.tile([P, maxB, P], bf16, tag="GB")
                        for c0 in range(0, b1_ - b0, GMAX):
                            gather_piece(GB, gtabB, idxB_s, b0,
                                         c0, min(c0 + GMAX, b1_ - b0))
                        for t in range(t0, t1):
                            emit_tile(t, P, GA, a0, GB, b0, sp, ap,
                                      consume_tile)

            def layer2(gtabA, gtabB, consume_tile):
                # All B-group chunks are gathered up-front (their hp table
                # block finishes AllGathering first: layer 1 emits B-side
                # segments before A-side ones), so the gather engine never
                # stalls behind the last collectives; A-chunks then stream
                # per segment, double-buffered.
                with (
                    tc.tile_pool(name="G2b", bufs=1) as gpb,
                    tc.tile_pool(name="G2", bufs=2) as gp,
                    tc.tile_pool(name="S2", bufs=4) as sp,
                    tc.tile_pool(name="agg2", bufs=2, space="PSUM") as ap,
                ):
                    GBall = gpb.tile([P, NBc, P], bf16, tag="GBall")
                    for c0 in range(0, NBc, GMAX):
                        gather_piece(GBall, gtabB, idxB_s, 0,
                                     c0, min(c0 + GMAX, NBc))
                    for t0, t1 in segs:
                        a0, a1 = offA[t0], offA[t1]
                        GA = gp.tile([P, maxA, P], bf16, tag="GA")
                        for c0 in range(0, a1 - a0, GMAX):
                            gather_piece(GA, gtabA, idxA_s, a0,
                                         c0, min(c0 + GMAX, a1 - a0))
                        for t in range(t0, t1):
                            emit_tile(t, 2 * OUT, GA, a0, GBall, 0, sp, ap,
                                      consume_tile)

            # ---------------- phase 1
            with (
                tc.tile_pool(name="sb1", bufs=3) as sb1,
                tc.tile_pool(name="hps", bufs=2, space="PSUM") as hps,
            ):
                def consume1(t, acc):
                    # agg[d, f] = hi + lo halves (DVE reads at most one PSUM
                    # input: stage hi through scalar first)
                    agg_sb = sb1.tile([P, IN], f32, tag="aggds")
                    nc.scalar.activation(agg_sb[:], acc[:, :IN], Act.Copy)
                    nc.vector.tensor_tensor(
                        out=agg_sb[:], in0=agg_sb[:], in1=acc[:, IN:2 * IN],
                        op=Alu.add)
                    # transpose to [f, d] for the f32 projections
                    aggT_ps = hps.tile([IN, P], f32, tag="aggT")
                    nc.tensor.transpose(aggT_ps[:], agg_sb[:], ident_s[:])
                    aggsb = sb1.tile([IN, P], f32, tag="aggsb")
                    nc.scalar.activation(aggsb[:], aggT_ps[:], Act.Copy)
                    hT_ps = hps.tile([HID, P], f32, tag="hT")
                    nc.tensor.matmul(hT_ps[:], lhsT=w1relT_s[:], rhs=aggsb[:],
                                     start=True, stop=False)
                    nc.tensor.matmul(hT_ps[:], lhsT=w1rootT_s[:],
                                     rhs=xT_own_s[:, t * P:(t + 1) * P],
                                     start=False, stop=True)
                    hT_sb = sb1.tile([HID, P], f32, tag="hTsb")
                    nc.scalar.activation(hT_sb[:], hT_ps[:], Act.Relu,
                                         bias=b1_s[:, 0:1])
                    hh_ps = hps.tile([P, 2 * OUT], f32, tag="hh")
                    nc.tensor.matmul(hh_ps[:], lhsT=hT_sb[:],
                                     rhs=w2bothT_s[:], start=True, stop=True)
                    nc.vector.tensor_tensor(
                        out=hroots[:, t * OUT:(t + 1) * OUT],
                        in0=hh_ps[:, OUT:], in1=b2_s[:], op=Alu.add)
                    # split hp into bf16 hi/lo packed row [hi|lo|garbage]
                    hp_pack = sb1.tile([P, P], bf16, tag="hppack")
                    nc.vector.tensor_copy(out=hp_pack[:, :OUT],
                                          in_=hh_ps[:, :OUT])
                    nc.vector.tensor_tensor(
                        out=hp_pack[:, OUT:2 * OUT], in0=hh_ps[:, :OUT],
                        in1=hp_pack[:, :OUT], op=Alu.subtract)
                    nc.sync.dma_start(
                        out=hp_local[t * P:(t + 1) * P, :],
                        in_=hp_pack[:])

                # Per-segment AllGather chunks fire as soon as each 7-tile
                # slice of hp_local is written, overlapping layer-1 compute.
                # hp_full rows are segment-major (seg_perm) so each chunk's
                # 8-rank output block is contiguous. Layer 1 runs the B-side
                # segments (table rows >= SPLIT, segs a_segs..) first so the
                # B-table collectives complete early -- layer 2 leads with
                # its B-group gathers and never waits at the phase boundary.
                segrows = cfg.SEG * P
                a_segs = cfg.SPLIT // (segrows * cfg.NCORES)
                seg_order = list(range(a_segs, len(segs))) + list(range(a_segs))
                fired = set()

                def allgather_seg(si):
                    nc.gpsimd.collective_compute(
                        "AllGather", mybir.AluOpType.bypass,
                        replica_groups=[list(range(cfg.NCORES))],
                        ins=[hp_local[si * segrows:(si + 1) * segrows, :]],
                        outs=[hp_full[si * segrows * cfg.NCORES:
                                      (si + 1) * segrows * cfg.NCORES, :]],
                    )

                def consume1_and_gather(t, acc):
                    consume1(t, acc)
                    si = t // cfg.SEG
                    if t == segs[si][1] - 1:
                        allgather_seg(si)
                        fired.add(si)

                layer1(x_tab[:cfg.SPLIT, :], x_tab[cfg.SPLIT:, :],
                       consume1_and_gather, seg_order)
                assert len(fired) == len(segs)

            # ---------------- phase 2
            with tc.tile_pool(name="sb2", bufs=3) as sb2:
                def consume2(t, acc):
                    o1 = sb2.tile([P, OUT], f32, tag="o1")
                    nc.scalar.activation(o1[:], acc[:, :OUT], Act.Copy)
                    nc.vector.tensor_tensor(
                        out=o1[:], in0=o1[:], in1=acc[:, OUT:2 * OUT],
                        op=Alu.add)
                    nc.vector.tensor_tensor(
                        out=o1[:], in0=o1[:],
                        in1=hroots[:, t * OUT:(t + 1) * OUT], op=Alu.add)
                    mx = sb2.tile([P, 1], f32, tag="mx")
                    nc.vector.reduce_max(out=mx[:], in_=o1[:],
                                         axis=mybir.AxisListType.X)
                    nmx = sb2.tile([P, 1], f32, tag="nmx")
                    nc.vector.tensor_scalar(nmx[:], mx[:], -1.0, None, Alu.mult)
                    esc = sb2.tile([P, OUT], f32, tag="esc")
                    ssum = sb2.tile([P, 1], f32, tag="ssum")
                    nc.scalar.activation(esc[:], o1[:], Act.Exp,
                                         bias=nmx[:, 0:1], accum_out=ssum[:])
                    lse = sb2.tile([P, 1], f32, tag="lse")
                    nc.scalar.activation(lse[:], ssum[:], Act.Ln)
                    shift = sb2.tile([P, 1], f32, tag="shift")
                    nc.vector.tensor_tensor(out=shift[:], in0=mx[:],
                                            in1=lse[:], op=Alu.add)
                    res = sb2.tile([P, OUT], f32, tag="res")
                    nc.vector.tensor_tensor(
                        out=res[:], in0=o1[:],
                        in1=shift[:, 0:1].to_broadcast([P, OUT]),
                        op=Alu.subtract)
                    nc.sync.dma_start(out=out_d[t * P:(t + 1) * P, :],
                                      in_=res[:])

                layer2(hp_full[:cfg.SPLIT, :], hp_full[cfg.SPLIT:, :],
                       consume2)

    nc.compile()
    return nc


# ---------------------------------------------------------------- runner

_CACHE = {}


def _get_program(cfg, kA, kB):
    key = (cfg.N, cfg.E, cfg.NCORES, cfg.TPC, tuple(kA), tuple(kB))
    if key not in _CACHE:
        _CACHE[key] = build(cfg, kA, kB)
    return _CACHE[key]


def run_gcn(inputs, cfg, trace=False):
    from concourse import bass_utils

    kA, kB, per_core = preprocess(inputs["x"], inputs["edge_index"], cfg)
    wts = make_weight_inputs(inputs["W1_rel"], inputs["b1"], inputs["W1_root"],
                             inputs["W2_rel"], inputs["b2"], inputs["W2_root"],
                             cfg)
    nc = _get_program(cfg, kA, kB)
    in_maps = []
    for c in range(cfg.NCORES):
        m = dict(per_core[c])
        m.update(wts)
        in_maps.append({k: m[k] for k in (
            "x_tab", "xT_own", "idxA", "idxB", "drA", "drB",
            "w1relT", "w1rootT", "b1", "w2bothT", "b2rep", "iota", "ident")})
    res = bass_utils.run_bass_kernel_spmd(
        nc, in_maps, core_ids=list(range(cfg.NCORES)), trace=trace)
    outs = [res.results[c]["out"] for c in range(cfg.NCORES)]
    full = np.concatenate(outs, axis=0)[:cfg.N]
    return full, res


def kernel(**inputs):
    out, _ = run_gcn(inputs, REAL_CFG)
    return out
